# revision 69
# baseline (speedup 1.0000x reference)
"""Trainium2 Bass kernel for MinibatchDiscrimination2d.

Full computation:
  x (32,128,64,64) --conv s4--> x_r (32,3,16,16)
  M = x_r @ T  -> (32, 8192, 16)
  dist[b1,b2,d] = sum_f |M[b1,d,f]-M[b2,d,f]|
  out[b,d] = sum_b2 exp(-dist) - 1 -> (32,32,16,16)
  out_a = deconv s4 (32,32,64,64); return concat([x, out_a], ch)

Split host/device: the tiny front-end (strided conv -> x_r, 32x768 = 98KB,
~0.1% of total FLOPs) and the x passthrough run on the host; the device
kernel does everything that dominates: streaming the 100M-parameter T, the
M matmul, the B^2-coupled pairwise-L1/exp stage, and the deconv.

Sharding over 8 cores: split the t*t=256 output spatial positions of the
D_OUT axis into 8 row-bands (2 of 16 t-rows per core). Each core gets a
(768, 1024, 16) slice of T (fp8 e4m3, x128 scale) and computes M/dist/out
for its band for ALL 32 samples (no cross-core coupling anywhere), then
deconvs its band into 8 of the 64 output rows.

fp8: T scaled x128, x_r scaled x2 -> M psum = 256*M, descaled on the
PSUM->SBUF copy. The M matmul uses DoubleRow (2 contraction rows per
partition, 768 = 3 pairs of 128).

Engine layout per (dgroup g, pair-chunk pc):
  M_b   = x_r @ T_g (TensorE, fp8 DR)        -> PSUM -> Mb bf16 (ACT copy)
  D     = sgn^T @ M_b (TensorE, zero-padded to 128 contraction rows so the
          PE array shows full cell activity and the HAM clock gate stays
          at 2.4 GHz)                        -> PSUM (128 pairs, (s, f))
  dist  = reduce_|.|_f(D) (DVE, the pacing stage: PSUM f32 reads are
          capped at 1x = ~110 G elem/s, ~78us total)
  E     = exp(-dist) (ACT), acc_g = E^T @ inc (TensorE)
Deconv: acc (f32) x block-diagonal wd (f32) per (v, q), contiguous stores.

Per-core d index:  s = (r*16 + j)*32 + ch   (r in 0..1, j in 0..15, ch in 0..31)
dgroup g = s // 128; partition p = s % 128 = (rj%4)*32 + ch.
"""

import numpy as np
import ml_dtypes

N_CORES = 8
B, IN_FLT, N = 32, 128, 64
K = 4
T_SP = 16
OC = 32
F = 16
D_IN = 768
DSH = 1024                 # d per core
NG = DSH // 128            # 8 dgroups
KP = 3                     # contraction pairs (768 = 3 * 256) for DoubleRow
T_SCALE = 128.0
XR_SCALE = 2.0
M_DESCALE = 1.0 / (T_SCALE * XR_SCALE)

_CACHE = {}


def _build_nc():
    import concourse.bacc as bacc
    import concourse.mybir as mybir
    import concourse.tile as tile

    f32 = mybir.dt.float32
    bf16 = mybir.dt.bfloat16
    fp8 = mybir.dt.float8e4
    AFT = mybir.ActivationFunctionType
    ALU = mybir.AluOpType
    DR = mybir.MatmulPerfMode.DoubleRow

    nc = bacc.Bacc("TRN2", target_bir_lowering=False, debug=False,
                   num_devices=N_CORES)

    # host-packed inputs (see _host_prep for layouts)
    xrT = nc.dram_tensor("xrT", [128, 6 * B], fp8, kind="ExternalInput")
    tsh = nc.dram_tensor("tsh", [NG * KP * 128, 4096], fp8, kind="ExternalInput")
    # wd as 16 zero-padded (128,128) blocks [q, v]: block (q,v) is nonzero
    # only on partitions [32q, 32q+32), so the deconv can contract acc's
    # q-th partition block using a full-128-partition matmul.
    wd = nc.dram_tensor("wd", [128, 2048], f32, kind="ExternalInput")
    # sgn zero-padded to 128 contraction rows (see engine notes above)
    sgn = nc.dram_tensor("sgn", [128, 512], bf16, kind="ExternalInput")
    inc = nc.dram_tensor("inc", [128, 128], bf16, kind="ExternalInput")
    # y layout [row(4r+u), oc, v, j, b]: deconv PSUM->SBUF copies and the
    # final stores are fully contiguous (one DMA per row); host untangles.
    y = nc.dram_tensor("y", [8, OC, 4, T_SP, B], bf16, kind="ExternalOutput")

    with tile.TileContext(nc) as tc:
        with tc.tile_pool(name="const", bufs=1) as constp, \
             tc.tile_pool(name="Tp", bufs=24) as Tp, \
             tc.tile_pool(name="work", bufs=2) as wp, \
             tc.tile_pool(name="persist", bufs=1) as pp, \
             tc.tile_pool(name="psb", bufs=2, space="PSUM") as psb, \
             tc.tile_pool(name="ps_m", bufs=2, space="PSUM") as ps_m, \
             tc.tile_pool(name="ps_acc", bufs=1, space="PSUM") as ps_acc, \
             tc.tile_pool(name="ps_junk", bufs=1, space="PSUM") as ps_junk:

            # sgn/inc first on the sync ring: the warmup junk matmuls (and
            # with them the whole tensor pipeline) wait on these two.
            sgn_sb = constp.tile([128, 512], bf16)
            nc.sync.dma_start(sgn_sb[:], sgn[:])
            inc_sb = constp.tile([128, 128], bf16)
            nc.sync.dma_start(inc_sb[:], inc[:])
            xrT_sb = constp.tile([128, 6 * B], fp8)
            nc.scalar.dma_start(xrT_sb[:], xrT[:])
            wd_sb = constp.tile([128, 2048], f32)
            nc.scalar.dma_start(wd_sb[:], wd[:])

            # junk-matmul helpers: full 128x128 array activity keeps the PE
            # HAM activity monitor above its warm threshold so the clock gate
            # stays at 2.4 GHz (it watches array cell activity, not busy%).
            # Outputs go to a dedicated PSUM bank that is never read.
            jpsum = ps_junk.tile([128, 512], f32, tag="junk")

            def _junk(n=1):
                for _ in range(n):
                    nc.tensor.matmul(jpsum[:], inc_sb[:],
                                     sgn_sb[:], start=True, stop=True)

            # Mb double-buffer: persistent 128-partition tiles whose rows
            # 32..127 stay zero forever (psD contraction zero-padding).
            Mbs = [pp.tile([128, 2048], bf16, tag=f"Mb{i}", name=f"Mb{i}")
                   for i in range(3)]
            for mb in Mbs:
                for q in range(1, 4):
                    nc.gpsimd.memset(mb[q * 32:(q + 1) * 32, :], 0.0)
            _junk(6)    # warm the PE before the pipeline fills

            xrT_r = xrT_sb[:].rearrange("p (c two b) -> p c two b", c=KP, two=2)

            acc = pp.tile([128, NG * B], f32)        # col = g*32 + b
            wd_v = wd_sb[:].rearrange("p (q v m) -> p q v m", q=4, v=4)

            def _deconv_r(r):
                # deconv reads acc directly (f32 x f32 matmul, no staging):
                # out[(u,oc), (g,q,b)] = wd_blk(q,v)^T @ acc[:, (g,b)]
                # with j = g*4 + q; one matmul per (v, q).
                yst = wp.tile([128, 2048], bf16, tag="yst")  # cols (v, j, b)
                for v in range(4):
                    if r == 0:   # mid-loop: keep clear of the hot psD slots
                        psdt = ps_junk.tile([128, 512], f32, tag="junk",
                                            name=f"psd_{r}_{v}")
                    else:        # tail: psD pipeline is idle, use its slots
                        psdt_t = psb.tile([128, 1024], f32, tag="big",
                                          name=f"psd_{r}_{v}")
                        psdt = psdt_t[:, :512]
                    psd_q = psdt[:].rearrange("p (g q b) -> p g q b", g=4, q=4)
                    for q in range(4):
                        nc.tensor.matmul(
                            psd_q[:, :, q, :],
                            wd_v[:, q, v],
                            acc[:, 4 * r * B:(4 * r + 4) * B],
                            start=True, stop=True)
                    if r == 1 and v % 2 == 0:
                        nc.vector.tensor_copy(yst[:, v * 512:(v + 1) * 512],
                                              psdt[:])
                    else:
                        nc.scalar.copy(yst[:, v * 512:(v + 1) * 512], psdt[:])
                for u in range(4):
                    nc.sync.dma_start(
                        y[4 * r + u],
                        yst[u * 32:(u + 1) * 32, :]
                        .rearrange("o (v j b) -> o v j b", v=4, j=T_SP))

            # all 24 T-chunk loads issued upfront (bufs=24: fully resident)
            Tall = []
            for g in range(NG):
                Ts = []
                for kp in range(KP):
                    Tt = Tp.tile([128, 4096], fp8, tag="T",
                                 name=f"T{g}_{kp}")
                    row = (g * KP + kp) * 128
                    nc.sync.dma_start(Tt[:], tsh[row:row + 128, :])
                    Ts.append(Tt)
                Tall.append(Ts)

            def _m_group(g, ncn):
                # M_b(g) ncn-quarter: 3 DoubleRow matmuls + descale copy
                psm = ps_m.tile([B, 512], f32, tag="mm", name=f"mm{g}_{ncn}")
                for kp in range(KP):
                    nc.tensor.matmul(
                        psm[:], xrT_r[:, kp],
                        Tall[g][kp][:].rearrange("p (two n) -> p two n", two=2)
                        [:, :, ncn * 512:(ncn + 1) * 512],
                        start=(kp == 0), stop=(kp == KP - 1), perf_mode=DR)
                nc.scalar.mul(Mbs[g % 3][:B, ncn * 512:(ncn + 1) * 512],
                              psm[:], M_DESCALE)

            # ---- main loop, software-pipelined: while (g, pc) runs its
            # pairwise stage, the tensor queue computes M for g+1 so dgroup
            # boundaries never stall the DVE reduce pipeline.
            for ncn in range(4):
                _m_group(0, ncn)
            for g in range(NG):
                Mb = Mbs[g % 3]                      # (128, (s, f)); rows 32+ zero
                accg = ps_acc.tile([128, B], f32, tag="accg")
                for pc in range(4):
                    dist = wp.tile([128, 128], f32, tag="dist")
                    for nh in range(2):
                        psD_t = psb.tile([128, 1024], f32, tag="big")
                        for nq in range(2):
                            ncn = nh * 2 + nq
                            nc.tensor.matmul(
                                psD_t[:, nq * 512:(nq + 1) * 512],
                                sgn_sb[:, pc * 128:(pc + 1) * 128],
                                Mb[:, ncn * 512:(ncn + 1) * 512],
                                start=True, stop=True)
                        nc.vector.tensor_reduce(
                            dist[:, nh * 64:(nh + 1) * 64],
                            psD_t[:].rearrange("p (s f) -> p s f", f=F),
                            axis=mybir.AxisListType.X, op=ALU.add,
                            apply_absolute_value=True)
                    Egp = wp.tile([128, 128], bf16, tag="E")
                    nc.scalar.activation(Egp[:], dist[:], AFT.Exp, scale=-1.0)
                    nc.tensor.matmul(
                        accg[:], Egp[:], inc_sb[:, pc * B:(pc + 1) * B],
                        start=(pc == 0), stop=(pc == 3))
                    if g + 1 < NG:
                        _m_group(g + 1, pc)
                    _junk(2)
                nc.scalar.copy(acc[:, g * B:(g + 1) * B], accg[:])
                if g in (NG // 2 - 1, NG - 1):
                    _deconv_r(g // (NG // 2))

    nc.finalize()
    return nc


def _host_prep(x, w_conv, T, w_deconv):
    """Host front-end (conv -> x_r, fp8 packing) + the 8 per-core inputs."""
    bf = ml_dtypes.bfloat16
    f8 = ml_dtypes.float8_e4m3

    def e4(v):
        return np.clip(v, -240.0, 240.0).astype(f8)

    # strided conv on host: x_r[b, o, i, j] (tiny: 32x3x16x16)
    xp = np.asarray(x, np.float32).reshape(B, IN_FLT, T_SP, K, T_SP, K)
    xr = np.einsum('bcirjs,ocrs->boij', xp, np.asarray(w_conv, np.float32),
                   optimize=True)
    # xrT[p, k*32 + b] = 2 * x_r[b, k*128 + p]  (fp8, matches T's x128 scale)
    xrf = xr.reshape(B, D_IN) * XR_SCALE
    xrT_host = e4(np.ascontiguousarray(xrf.T.reshape(6, 128, B)
                                       .transpose(1, 0, 2).reshape(128, 6 * B)))

    # deconv weights: block (q, v) nonzero only on partitions [32q, 32q+32),
    # wd_host[32q+ic, q, v, u*32+oc] = w_deconv[oc, ic, u, v]
    wd0 = np.transpose(w_deconv, (1, 3, 2, 0)).reshape(OC, 4, 128)  # [c, v, m]
    wdq = np.zeros((128, 4, 4, 128), np.float32)
    for q in range(4):
        wdq[32 * q:32 * (q + 1), q] = wd0
    wd_host = wdq.reshape(128, 2048)

    # pairwise sign matrix (b1 < b2, 496 pairs padded to 512) and incidence
    # (zero-padded to 128 contraction rows for full PE activity)
    pairs = [(a, b) for a in range(B) for b in range(a + 1, B)]
    sgn_host = np.zeros((128, 512), np.float32)
    inc_host = np.zeros((128, 128), np.float32)
    for p, (a, b) in enumerate(pairs):
        sgn_host[a, p] = 1.0
        sgn_host[b, p] = -1.0
        inc_host[p % 128, (p // 128) * B + a] = 1.0
        inc_host[p % 128, (p // 128) * B + b] = 1.0
    sgn_host = sgn_host.astype(bf)
    inc_host = inc_host.astype(bf)

    Tq = e4(np.asarray(T, np.float32) * T_SCALE).reshape(
        D_IN, OC, T_SP, T_SP, F)

    in_maps = []
    for c in range(N_CORES):
        # T shard: i rows 2c, 2c+1; column order s=(r*16+j)*32+ch, then f
        tslice = Tq[:, :, 2 * c:2 * c + 2, :, :]            # (768, ch, r, j, f)
        tshard = np.ascontiguousarray(
            np.transpose(tslice, (0, 2, 3, 1, 4)).reshape(D_IN, DSH * F))
        # DoubleRow pack: rows (g, kp, p), cols (two, n)
        t3 = tshard.reshape(KP, 2, 128, NG, 2048)
        tpk = np.ascontiguousarray(t3.transpose(3, 0, 2, 1, 4)).reshape(
            NG * KP * 128, 4096)
        in_maps.append({
            "xrT": xrT_host,
            "tsh": tpk,
            "wd": wd_host,
            "sgn": sgn_host,
            "inc": inc_host,
        })
    return in_maps


def _get_nc():
    if "nc" not in _CACHE:
        _CACHE["nc"] = _build_nc()
    return _CACHE["nc"]


def run(inputs, trace=False, trace_kwargs=None):
    """Run on hardware; returns (full_output, BassKernelResults)."""
    from concourse.bass_utils import run_bass_kernel_spmd
    nc = _get_nc()
    in_maps = _host_prep(inputs["x"], inputs["w_conv"], inputs["T"],
                         inputs["w_deconv"])
    res = run_bass_kernel_spmd(nc, in_maps, list(range(N_CORES)), trace=trace,
                               **(trace_kwargs or {}))
    x = np.asarray(inputs["x"], dtype=np.float32)
    full = np.empty((B, IN_FLT + OC, N, N), np.float32)
    full[:, :IN_FLT] = x
    for c in range(N_CORES):
        ya = res.results[c]["y"].astype(np.float32)   # [row, oc, v, j, b]
        # full[b, 128+oc, 8c+row, 4j+v] = ya[row, oc, v, j, b]
        yh = np.transpose(ya, (4, 1, 0, 3, 2)).reshape(B, OC, 8, N)
        full[:, IN_FLT:, 8 * c:8 * (c + 1), :] = yh
    return full, res


def kernel(**inputs) -> np.ndarray:
    out, _ = run(inputs, trace=False)
    return out


# revision 73
# speedup vs baseline: 1.0271x; 1.0271x over previous
"""Trainium2 Bass kernel for MinibatchDiscrimination2d.

Full computation:
  x (32,128,64,64) --conv s4--> x_r (32,3,16,16)
  M = x_r @ T  -> (32, 8192, 16)
  dist[b1,b2,d] = sum_f |M[b1,d,f]-M[b2,d,f]|
  out[b,d] = sum_b2 exp(-dist) - 1 -> (32,32,16,16)
  out_a = deconv s4 (32,32,64,64); return concat([x, out_a], ch)

Split host/device: the tiny front-end (strided conv -> x_r, 32x768 = 98KB,
~0.1% of total FLOPs) and the x passthrough run on the host; the device
kernel does everything that dominates: streaming the 100M-parameter T, the
M matmul, the B^2-coupled pairwise-L1/exp stage, and the deconv.

Sharding over 8 cores: split the t*t=256 output spatial positions of the
D_OUT axis into 8 row-bands (2 of 16 t-rows per core). Each core gets a
(768, 1024, 16) slice of T (fp8 e4m3, x128 scale) and computes M/dist/out
for its band for ALL 32 samples (no cross-core coupling anywhere), then
deconvs its band into 8 of the 64 output rows.

fp8: T scaled x128, x_r scaled x2 -> M psum = 256*M, descaled on the
PSUM->SBUF copy. The M matmul uses DoubleRow (2 contraction rows per
partition, 768 = 3 pairs of 128).

Engine layout per (dgroup g, pair-chunk pc):
  M_b   = x_r @ T_g (TensorE, fp8 DR)        -> PSUM -> Mb bf16 (ACT copy)
  D     = sgn^T @ M_b (TensorE, zero-padded to 128 contraction rows so the
          PE array shows full cell activity and the HAM clock gate stays
          at 2.4 GHz)                        -> PSUM (128 pairs, (s, f))
  dist  = reduce_|.|_f(D) (DVE, the pacing stage: PSUM f32 reads are
          capped at 1x = ~110 G elem/s, ~78us total)
  E     = exp(-dist) (ACT), acc_g = E^T @ inc (TensorE)
Deconv: acc (f32) x block-diagonal wd (f32) per (v, q), contiguous stores.

Per-core d index:  s = (r*16 + j)*32 + ch   (r in 0..1, j in 0..15, ch in 0..31)
dgroup g = s // 128; partition p = s % 128 = (rj%4)*32 + ch.
"""

import numpy as np
import ml_dtypes

N_CORES = 8
B, IN_FLT, N = 32, 128, 64
K = 4
T_SP = 16
OC = 32
F = 16
D_IN = 768
DSH = 1024                 # d per core
NG = DSH // 128            # 8 dgroups
KP = 3                     # contraction pairs (768 = 3 * 256) for DoubleRow
T_SCALE = 128.0
XR_SCALE = 2.0
M_DESCALE = 1.0 / (T_SCALE * XR_SCALE)

_CACHE = {}


def _build_nc():
    import concourse.bacc as bacc
    import concourse.mybir as mybir
    import concourse.tile as tile

    f32 = mybir.dt.float32
    bf16 = mybir.dt.bfloat16
    fp8 = mybir.dt.float8e4
    AFT = mybir.ActivationFunctionType
    ALU = mybir.AluOpType
    DR = mybir.MatmulPerfMode.DoubleRow

    nc = bacc.Bacc("TRN2", target_bir_lowering=False, debug=False,
                   num_devices=N_CORES)

    # host-packed inputs (see _host_prep for layouts)
    xrT = nc.dram_tensor("xrT", [128, 6 * B], fp8, kind="ExternalInput")
    tsh = nc.dram_tensor("tsh", [NG * KP * 128, 4096], fp8, kind="ExternalInput")
    # wd as 16 zero-padded (128,128) blocks [q, v]: block (q,v) is nonzero
    # only on partitions [32q, 32q+32), so the deconv can contract acc's
    # q-th partition block using a full-128-partition matmul.
    wd = nc.dram_tensor("wd", [128, 2048], f32, kind="ExternalInput")
    # sgn zero-padded to 128 contraction rows (see engine notes above)
    sgn = nc.dram_tensor("sgn", [128, 512], bf16, kind="ExternalInput")
    inc = nc.dram_tensor("inc", [128, 128], bf16, kind="ExternalInput")
    # y layout [row(4r+u), oc, v, j, b]: deconv PSUM->SBUF copies and the
    # final stores are fully contiguous (one DMA per row); host untangles.
    y = nc.dram_tensor("y", [8, OC, 4, T_SP, B], bf16, kind="ExternalOutput")

    with tile.TileContext(nc) as tc:
        with tc.tile_pool(name="const", bufs=1) as constp, \
             tc.tile_pool(name="Tp", bufs=24) as Tp, \
             tc.tile_pool(name="work", bufs=2) as wp, \
             tc.tile_pool(name="persist", bufs=1) as pp, \
             tc.tile_pool(name="psb", bufs=2, space="PSUM") as psb, \
             tc.tile_pool(name="ps_m", bufs=2, space="PSUM") as ps_m, \
             tc.tile_pool(name="ps_acc", bufs=1, space="PSUM") as ps_acc, \
             tc.tile_pool(name="ps_junk", bufs=1, space="PSUM") as ps_junk:

            sgn_sb = constp.tile([128, 512], bf16)
            nc.scalar.dma_start(sgn_sb[:], sgn[:])
            inc_sb = constp.tile([128, 128], bf16)
            nc.scalar.dma_start(inc_sb[:], inc[:])
            xrT_sb = constp.tile([128, 6 * B], fp8)
            nc.scalar.dma_start(xrT_sb[:], xrT[:])
            wd_sb = constp.tile([128, 2048], f32)
            nc.scalar.dma_start(wd_sb[:], wd[:])

            # junk-matmul helpers: full 128x128 array activity keeps the PE
            # HAM activity monitor above its warm threshold so the clock gate
            # stays at 2.4 GHz (it watches array cell activity, not busy%).
            # Outputs go to a dedicated PSUM bank that is never read.
            jpsum = ps_junk.tile([128, 512], f32, tag="junk")

            def _junk(n=1):
                for _ in range(n):
                    nc.tensor.matmul(jpsum[:], inc_sb[:],
                                     sgn_sb[:], start=True, stop=True)

            # Mb double-buffer: persistent 128-partition tiles whose rows
            # 32..127 stay zero forever (psD contraction zero-padding).
            Mbs = [pp.tile([128, 2048], bf16, tag=f"Mb{i}", name=f"Mb{i}")
                   for i in range(2)]
            for mb in Mbs:
                for q in range(1, 4):
                    nc.gpsimd.memset(mb[q * 32:(q + 1) * 32, :], 0.0)
            _junk(16)   # warm the PE before the pipeline fills

            xrT_r = xrT_sb[:].rearrange("p (c two b) -> p c two b", c=KP, two=2)

            acc = pp.tile([128, NG * B], f32)        # col = g*32 + b
            wd_v = wd_sb[:].rearrange("p (q v m) -> p q v m", q=4, v=4)

            def _deconv_r(r):
                # deconv reads acc directly (f32 x f32 matmul, no staging):
                # out[(u,oc), (g,q,b)] = wd_blk(q,v)^T @ acc[:, (g,b)]
                # with j = g*4 + q; one matmul per (v, q).
                yst = wp.tile([128, 2048], bf16, tag="yst")  # cols (v, j, b)
                for v in range(4):
                    if r == 0:   # mid-loop: keep clear of the hot psD slots
                        psdt = ps_junk.tile([128, 512], f32, tag="junk",
                                            name=f"psd_{r}_{v}")
                    else:        # tail: psD pipeline is idle, use its slots
                        psdt_t = psb.tile([128, 1024], f32, tag="big",
                                          name=f"psd_{r}_{v}")
                        psdt = psdt_t[:, :512]
                    psd_q = psdt[:].rearrange("p (g q b) -> p g q b", g=4, q=4)
                    for q in range(4):
                        nc.tensor.matmul(
                            psd_q[:, :, q, :],
                            wd_v[:, q, v],
                            acc[:, 4 * r * B:(4 * r + 4) * B],
                            start=True, stop=True)
                    if r == 1 and v % 2 == 0:
                        nc.vector.tensor_copy(yst[:, v * 512:(v + 1) * 512],
                                              psdt[:])
                    else:
                        nc.scalar.copy(yst[:, v * 512:(v + 1) * 512], psdt[:])
                for u in range(4):
                    nc.sync.dma_start(
                        y[4 * r + u],
                        yst[u * 32:(u + 1) * 32, :]
                        .rearrange("o (v j b) -> o v j b", v=4, j=T_SP))

            # ---- main loop: stages fused per dgroup g
            for g in range(NG):
                Ts = []
                for kp in range(KP):
                    Tt = Tp.tile([128, 4096], fp8, tag="T")
                    row = (g * KP + kp) * 128
                    nc.sync.dma_start(Tt[:], tsh[row:row + 128, :])
                    Ts.append(Tt)
                Mb = Mbs[g % 2]                      # (128, (s, f)); rows 32+ zero
                for ncn in range(4):
                    psm = ps_m.tile([B, 512], f32, tag="mm")
                    for kp in range(KP):
                        nc.tensor.matmul(
                            psm[:], xrT_r[:, kp],
                            Ts[kp][:].rearrange("p (two n) -> p two n", two=2)
                            [:, :, ncn * 512:(ncn + 1) * 512],
                            start=(kp == 0), stop=(kp == KP - 1), perf_mode=DR)
                    nc.scalar.mul(Mb[:B, ncn * 512:(ncn + 1) * 512], psm[:], M_DESCALE)
                accg = ps_acc.tile([128, B], f32, tag="accg")
                for pc in range(4):
                    dist = wp.tile([128, 128], f32, tag="dist")
                    for nh in range(2):
                        psD_t = psb.tile([128, 1024], f32, tag="big")
                        for nq in range(2):
                            ncn = nh * 2 + nq
                            nc.tensor.matmul(
                                psD_t[:, nq * 512:(nq + 1) * 512],
                                sgn_sb[:, pc * 128:(pc + 1) * 128],
                                Mb[:, ncn * 512:(ncn + 1) * 512],
                                start=True, stop=True)
                        nc.vector.tensor_reduce(
                            dist[:, nh * 64:(nh + 1) * 64],
                            psD_t[:].rearrange("p (s f) -> p s f", f=F),
                            axis=mybir.AxisListType.X, op=ALU.add,
                            apply_absolute_value=True)
                    Egp = wp.tile([128, 128], bf16, tag="E")
                    nc.scalar.activation(Egp[:], dist[:], AFT.Exp, scale=-1.0)
                    nc.tensor.matmul(
                        accg[:], Egp[:], inc_sb[:, pc * B:(pc + 1) * B],
                        start=(pc == 0), stop=(pc == 3))
                    _junk(2)
                nc.scalar.copy(acc[:, g * B:(g + 1) * B], accg[:])
                if g in (NG // 2 - 1, NG - 1):
                    _deconv_r(g // (NG // 2))

    nc.finalize()
    return nc


def _host_prep(x, w_conv, T, w_deconv):
    """Host front-end (conv -> x_r, fp8 packing) + the 8 per-core inputs."""
    bf = ml_dtypes.bfloat16
    f8 = ml_dtypes.float8_e4m3

    def e4(v):
        return np.clip(v, -240.0, 240.0).astype(f8)

    # strided conv on host: x_r[b, o, i, j] (tiny: 32x3x16x16)
    xp = np.asarray(x, np.float32).reshape(B, IN_FLT, T_SP, K, T_SP, K)
    xr = np.einsum('bcirjs,ocrs->boij', xp, np.asarray(w_conv, np.float32),
                   optimize=True)
    # xrT[p, k*32 + b] = 2 * x_r[b, k*128 + p]  (fp8, matches T's x128 scale)
    xrf = xr.reshape(B, D_IN) * XR_SCALE
    xrT_host = e4(np.ascontiguousarray(xrf.T.reshape(6, 128, B)
                                       .transpose(1, 0, 2).reshape(128, 6 * B)))

    # deconv weights: block (q, v) nonzero only on partitions [32q, 32q+32),
    # wd_host[32q+ic, q, v, u*32+oc] = w_deconv[oc, ic, u, v]
    wd0 = np.transpose(w_deconv, (1, 3, 2, 0)).reshape(OC, 4, 128)  # [c, v, m]
    wdq = np.zeros((128, 4, 4, 128), np.float32)
    for q in range(4):
        wdq[32 * q:32 * (q + 1), q] = wd0
    wd_host = wdq.reshape(128, 2048)

    # pairwise sign matrix (b1 < b2, 496 pairs padded to 512) and incidence
    # (zero-padded to 128 contraction rows for full PE activity)
    pairs = [(a, b) for a in range(B) for b in range(a + 1, B)]
    sgn_host = np.zeros((128, 512), np.float32)
    inc_host = np.zeros((128, 128), np.float32)
    for p, (a, b) in enumerate(pairs):
        sgn_host[a, p] = 1.0
        sgn_host[b, p] = -1.0
        inc_host[p % 128, (p // 128) * B + a] = 1.0
        inc_host[p % 128, (p // 128) * B + b] = 1.0
    sgn_host = sgn_host.astype(bf)
    inc_host = inc_host.astype(bf)

    Tq = e4(np.asarray(T, np.float32) * T_SCALE).reshape(
        D_IN, OC, T_SP, T_SP, F)

    in_maps = []
    for c in range(N_CORES):
        # T shard: i rows 2c, 2c+1; column order s=(r*16+j)*32+ch, then f
        tslice = Tq[:, :, 2 * c:2 * c + 2, :, :]            # (768, ch, r, j, f)
        tshard = np.ascontiguousarray(
            np.transpose(tslice, (0, 2, 3, 1, 4)).reshape(D_IN, DSH * F))
        # DoubleRow pack: rows (g, kp, p), cols (two, n)
        t3 = tshard.reshape(KP, 2, 128, NG, 2048)
        tpk = np.ascontiguousarray(t3.transpose(3, 0, 2, 1, 4)).reshape(
            NG * KP * 128, 4096)
        in_maps.append({
            "xrT": xrT_host,
            "tsh": tpk,
            "wd": wd_host,
            "sgn": sgn_host,
            "inc": inc_host,
        })
    return in_maps


def _get_nc():
    if "nc" not in _CACHE:
        _CACHE["nc"] = _build_nc()
    return _CACHE["nc"]


def run(inputs, trace=False, trace_kwargs=None):
    """Run on hardware; returns (full_output, BassKernelResults)."""
    from concourse.bass_utils import run_bass_kernel_spmd
    nc = _get_nc()
    in_maps = _host_prep(inputs["x"], inputs["w_conv"], inputs["T"],
                         inputs["w_deconv"])
    res = run_bass_kernel_spmd(nc, in_maps, list(range(N_CORES)), trace=trace,
                               **(trace_kwargs or {}))
    x = np.asarray(inputs["x"], dtype=np.float32)
    full = np.empty((B, IN_FLT + OC, N, N), np.float32)
    full[:, :IN_FLT] = x
    for c in range(N_CORES):
        ya = res.results[c]["y"].astype(np.float32)   # [row, oc, v, j, b]
        # full[b, 128+oc, 8c+row, 4j+v] = ya[row, oc, v, j, b]
        yh = np.transpose(ya, (4, 1, 0, 3, 2)).reshape(B, OC, 8, N)
        full[:, IN_FLT:, 8 * c:8 * (c + 1), :] = yh
    return full, res


def kernel(**inputs) -> np.ndarray:
    out, _ = run(inputs, trace=False)
    return out


# revision 76
# speedup vs baseline: 1.0360x; 1.0087x over previous
"""Trainium2 Bass kernel for MinibatchDiscrimination2d.

Full computation:
  x (32,128,64,64) --conv s4--> x_r (32,3,16,16)
  M = x_r @ T  -> (32, 8192, 16)
  dist[b1,b2,d] = sum_f |M[b1,d,f]-M[b2,d,f]|
  out[b,d] = sum_b2 exp(-dist) - 1 -> (32,32,16,16)
  out_a = deconv s4 (32,32,64,64); return concat([x, out_a], ch)

Split host/device: the tiny front-end (strided conv -> x_r, 32x768 = 98KB,
~0.1% of total FLOPs) and the x passthrough run on the host; the device
kernel does everything that dominates: streaming the 100M-parameter T, the
M matmul, the B^2-coupled pairwise-L1/exp stage, and the deconv.

Sharding over 8 cores: split the t*t=256 output spatial positions of the
D_OUT axis into 8 row-bands (2 of 16 t-rows per core). Each core gets a
(768, 1024, 16) slice of T (fp8 e4m3, x128 scale) and computes M/dist/out
for its band for ALL 32 samples (no cross-core coupling anywhere), then
deconvs its band into 8 of the 64 output rows.

fp8: T scaled x128, x_r scaled x2 -> M psum = 256*M, descaled on the
PSUM->SBUF copy. The M matmul uses DoubleRow (2 contraction rows per
partition, 768 = 3 pairs of 128).

Engine layout per (dgroup g, pair-chunk pc):
  M_b   = x_r @ T_g (TensorE, fp8 DR)        -> PSUM -> Mb bf16 (ACT copy)
  D     = sgn^T @ M_b (TensorE, zero-padded to 128 contraction rows so the
          PE array shows full cell activity and the HAM clock gate stays
          at 2.4 GHz)                        -> PSUM (128 pairs, (s, f))
  dist  = reduce_|.|_f(D) (DVE, the pacing stage: PSUM f32 reads are
          capped at 1x = ~110 G elem/s, ~78us total)
  E     = exp(-dist) (ACT), acc_g = E^T @ inc (TensorE)
Deconv: acc (f32) x block-diagonal wd (f32) per (v, q), contiguous stores.

Per-core d index:  s = (r*16 + j)*32 + ch   (r in 0..1, j in 0..15, ch in 0..31)
dgroup g = s // 128; partition p = s % 128 = (rj%4)*32 + ch.
"""

import numpy as np
import ml_dtypes

N_CORES = 8
B, IN_FLT, N = 32, 128, 64
K = 4
T_SP = 16
OC = 32
F = 16
D_IN = 768
DSH = 1024                 # d per core
NG = DSH // 128            # 8 dgroups
KP = 3                     # contraction pairs (768 = 3 * 256) for DoubleRow
T_SCALE = 128.0
XR_SCALE = 2.0
M_DESCALE = 1.0 / (T_SCALE * XR_SCALE)

_CACHE = {}


def _build_nc():
    import concourse.bacc as bacc
    import concourse.mybir as mybir
    import concourse.tile as tile

    f32 = mybir.dt.float32
    bf16 = mybir.dt.bfloat16
    fp8 = mybir.dt.float8e4
    AFT = mybir.ActivationFunctionType
    ALU = mybir.AluOpType
    DR = mybir.MatmulPerfMode.DoubleRow

    nc = bacc.Bacc("TRN2", target_bir_lowering=False, debug=False,
                   num_devices=N_CORES)

    # host-packed inputs (see _host_prep for layouts)
    xrT = nc.dram_tensor("xrT", [128, 6 * B], fp8, kind="ExternalInput")
    tsh = nc.dram_tensor("tsh", [NG * KP * 128, 4096], fp8, kind="ExternalInput")
    # wd as 16 zero-padded (128,128) blocks [q, v]: block (q,v) is nonzero
    # only on partitions [32q, 32q+32), so the deconv can contract acc's
    # q-th partition block using a full-128-partition matmul.
    wd = nc.dram_tensor("wd", [128, 2048], f32, kind="ExternalInput")
    # sgn zero-padded to 128 contraction rows (see engine notes above)
    sgn = nc.dram_tensor("sgn", [128, 512], bf16, kind="ExternalInput")
    inc = nc.dram_tensor("inc", [128, 128], bf16, kind="ExternalInput")
    # y layout [row(4r+u), oc, v, j, b]: deconv PSUM->SBUF copies and the
    # final stores are fully contiguous (one DMA per row); host untangles.
    y = nc.dram_tensor("y", [8, OC, 4, T_SP, B], bf16, kind="ExternalOutput")

    with tile.TileContext(nc) as tc:
        with tc.tile_pool(name="const", bufs=1) as constp, \
             tc.tile_pool(name="Tp", bufs=24) as Tp, \
             tc.tile_pool(name="work", bufs=2) as wp, \
             tc.tile_pool(name="persist", bufs=1) as pp, \
             tc.tile_pool(name="psb", bufs=2, space="PSUM") as psb, \
             tc.tile_pool(name="ps_m", bufs=2, space="PSUM") as ps_m, \
             tc.tile_pool(name="ps_acc", bufs=1, space="PSUM") as ps_acc, \
             tc.tile_pool(name="ps_junk", bufs=1, space="PSUM") as ps_junk:

            sgn_sb = constp.tile([128, 512], bf16)
            nc.scalar.dma_start(sgn_sb[:], sgn[:])
            inc_sb = constp.tile([128, 128], bf16)
            nc.scalar.dma_start(inc_sb[:], inc[:])
            xrT_sb = constp.tile([128, 6 * B], fp8)
            nc.scalar.dma_start(xrT_sb[:], xrT[:])
            wd_sb = constp.tile([128, 2048], f32)
            nc.scalar.dma_start(wd_sb[:], wd[:])

            # junk-matmul helpers: full 128x128 array activity keeps the PE
            # HAM activity monitor above its warm threshold so the clock gate
            # stays at 2.4 GHz (it watches array cell activity, not busy%).
            # Outputs go to a dedicated PSUM bank that is never read.
            jpsum = ps_junk.tile([128, 512], f32, tag="junk")

            def _junk(n=1):
                for _ in range(n):
                    nc.tensor.matmul(jpsum[:], inc_sb[:],
                                     sgn_sb[:], start=True, stop=True)

            # Mb double-buffer: persistent 128-partition tiles whose rows
            # 32..127 stay zero forever (psD contraction zero-padding).
            Mbs = [pp.tile([128, 2048], bf16, tag=f"Mb{i}", name=f"Mb{i}")
                   for i in range(2)]
            for mb in Mbs:
                for q in range(1, 4):
                    nc.gpsimd.memset(mb[q * 32:(q + 1) * 32, :], 0.0)
            _junk(16)   # warm the PE before the pipeline fills

            xrT_r = xrT_sb[:].rearrange("p (c two b) -> p c two b", c=KP, two=2)

            acc = pp.tile([128, NG * B], f32)        # col = g*32 + b
            wd_v = wd_sb[:].rearrange("p (q v m) -> p q v m", q=4, v=4)

            def _deconv_r(r):
                # deconv reads acc directly (f32 x f32 matmul, no staging):
                # out[(u,oc), (g,q,b)] = wd_blk(q,v)^T @ acc[:, (g,b)]
                # with j = g*4 + q; one matmul per (v, q).
                yst = wp.tile([128, 2048], bf16, tag="yst")  # cols (v, j, b)
                for v in range(4):
                    if r == 0:   # mid-loop: keep clear of the hot psD slots
                        psdt = ps_junk.tile([128, 512], f32, tag="junk",
                                            name=f"psd_{r}_{v}")
                    else:        # tail: psD pipeline is idle, use its slots
                        psdt_t = psb.tile([128, 1024], f32, tag="big",
                                          name=f"psd_{r}_{v}")
                        psdt = psdt_t[:, :512]
                    psd_q = psdt[:].rearrange("p (g q b) -> p g q b", g=4, q=4)
                    for q in range(4):
                        nc.tensor.matmul(
                            psd_q[:, :, q, :],
                            wd_v[:, q, v],
                            acc[:, 4 * r * B:(4 * r + 4) * B],
                            start=True, stop=True)
                    if r == 1 and v % 2 == 0:
                        nc.vector.tensor_copy(yst[:, v * 512:(v + 1) * 512],
                                              psdt[:])
                    else:
                        nc.scalar.copy(yst[:, v * 512:(v + 1) * 512], psdt[:])
                for u in range(4):
                    nc.sync.dma_start(
                        y[4 * r + u],
                        yst[u * 32:(u + 1) * 32, :]
                        .rearrange("o (v j b) -> o v j b", v=4, j=T_SP))

            # E-accumulation is emitted one pc-slot late so the acc matmul
            # (which depends on its own pc's reduce->exp chain) never sits in
            # the in-order tensor queue ahead of the next psD group.
            pend = []
            accg_box = [None]

            def _flush_acc():
                if not pend:
                    return
                fg, fpc, fE = pend.pop(0)
                if fpc == 0:
                    accg_box[0] = ps_acc.tile([128, B], f32, tag="accg",
                                              name=f"accg{fg}")
                nc.tensor.matmul(
                    accg_box[0][:], fE[:], inc_sb[:, fpc * B:(fpc + 1) * B],
                    start=(fpc == 0), stop=(fpc == 3))
                if fpc == 3:
                    nc.scalar.copy(acc[:, fg * B:(fg + 1) * B], accg_box[0][:])
                    if fg in (NG // 2 - 1, NG - 1):
                        _deconv_r(fg // (NG // 2))

            # ---- main loop: stages fused per dgroup g
            for g in range(NG):
                Ts = []
                for kp in range(KP):
                    Tt = Tp.tile([128, 4096], fp8, tag="T")
                    row = (g * KP + kp) * 128
                    nc.sync.dma_start(Tt[:], tsh[row:row + 128, :])
                    Ts.append(Tt)
                Mb = Mbs[g % 2]                      # (128, (s, f)); rows 32+ zero
                for ncn in range(4):
                    psm = ps_m.tile([B, 512], f32, tag="mm")
                    for kp in range(KP):
                        nc.tensor.matmul(
                            psm[:], xrT_r[:, kp],
                            Ts[kp][:].rearrange("p (two n) -> p two n", two=2)
                            [:, :, ncn * 512:(ncn + 1) * 512],
                            start=(kp == 0), stop=(kp == KP - 1), perf_mode=DR)
                    nc.scalar.mul(Mb[:B, ncn * 512:(ncn + 1) * 512], psm[:], M_DESCALE)
                for pc in range(4):
                    dist = wp.tile([128, 128], f32, tag="dist")
                    for nh in range(2):
                        psD_t = psb.tile([128, 1024], f32, tag="big")
                        for nq in range(2):
                            ncn = nh * 2 + nq
                            nc.tensor.matmul(
                                psD_t[:, nq * 512:(nq + 1) * 512],
                                sgn_sb[:, pc * 128:(pc + 1) * 128],
                                Mb[:, ncn * 512:(ncn + 1) * 512],
                                start=True, stop=True)
                        if nh == 0:
                            _flush_acc()   # previous pc's E-accumulation
                        nc.vector.tensor_reduce(
                            dist[:, nh * 64:(nh + 1) * 64],
                            psD_t[:].rearrange("p (s f) -> p s f", f=F),
                            axis=mybir.AxisListType.X, op=ALU.add,
                            apply_absolute_value=True)
                    Egp = wp.tile([128, 128], bf16, tag="E")
                    nc.scalar.activation(Egp[:], dist[:], AFT.Exp, scale=-1.0)
                    pend.append((g, pc, Egp))
                    _junk(2)
            _flush_acc()   # final (g=7, pc=3) accumulation + copy + deconv

    nc.finalize()
    return nc


def _host_prep(x, w_conv, T, w_deconv):
    """Host front-end (conv -> x_r, fp8 packing) + the 8 per-core inputs."""
    bf = ml_dtypes.bfloat16
    f8 = ml_dtypes.float8_e4m3

    def e4(v):
        return np.clip(v, -240.0, 240.0).astype(f8)

    # strided conv on host: x_r[b, o, i, j] (tiny: 32x3x16x16)
    xp = np.asarray(x, np.float32).reshape(B, IN_FLT, T_SP, K, T_SP, K)
    xr = np.einsum('bcirjs,ocrs->boij', xp, np.asarray(w_conv, np.float32),
                   optimize=True)
    # xrT[p, k*32 + b] = 2 * x_r[b, k*128 + p]  (fp8, matches T's x128 scale)
    xrf = xr.reshape(B, D_IN) * XR_SCALE
    xrT_host = e4(np.ascontiguousarray(xrf.T.reshape(6, 128, B)
                                       .transpose(1, 0, 2).reshape(128, 6 * B)))

    # deconv weights: block (q, v) nonzero only on partitions [32q, 32q+32),
    # wd_host[32q+ic, q, v, u*32+oc] = w_deconv[oc, ic, u, v]
    wd0 = np.transpose(w_deconv, (1, 3, 2, 0)).reshape(OC, 4, 128)  # [c, v, m]
    wdq = np.zeros((128, 4, 4, 128), np.float32)
    for q in range(4):
        wdq[32 * q:32 * (q + 1), q] = wd0
    wd_host = wdq.reshape(128, 2048)

    # pairwise sign matrix (b1 < b2, 496 pairs padded to 512) and incidence
    # (zero-padded to 128 contraction rows for full PE activity)
    pairs = [(a, b) for a in range(B) for b in range(a + 1, B)]
    sgn_host = np.zeros((128, 512), np.float32)
    inc_host = np.zeros((128, 128), np.float32)
    for p, (a, b) in enumerate(pairs):
        sgn_host[a, p] = 1.0
        sgn_host[b, p] = -1.0
        inc_host[p % 128, (p // 128) * B + a] = 1.0
        inc_host[p % 128, (p // 128) * B + b] = 1.0
    sgn_host = sgn_host.astype(bf)
    inc_host = inc_host.astype(bf)

    Tq = e4(np.asarray(T, np.float32) * T_SCALE).reshape(
        D_IN, OC, T_SP, T_SP, F)

    in_maps = []
    for c in range(N_CORES):
        # T shard: i rows 2c, 2c+1; column order s=(r*16+j)*32+ch, then f
        tslice = Tq[:, :, 2 * c:2 * c + 2, :, :]            # (768, ch, r, j, f)
        tshard = np.ascontiguousarray(
            np.transpose(tslice, (0, 2, 3, 1, 4)).reshape(D_IN, DSH * F))
        # DoubleRow pack: rows (g, kp, p), cols (two, n)
        t3 = tshard.reshape(KP, 2, 128, NG, 2048)
        tpk = np.ascontiguousarray(t3.transpose(3, 0, 2, 1, 4)).reshape(
            NG * KP * 128, 4096)
        in_maps.append({
            "xrT": xrT_host,
            "tsh": tpk,
            "wd": wd_host,
            "sgn": sgn_host,
            "inc": inc_host,
        })
    return in_maps


def _get_nc():
    if "nc" not in _CACHE:
        _CACHE["nc"] = _build_nc()
    return _CACHE["nc"]


def run(inputs, trace=False, trace_kwargs=None):
    """Run on hardware; returns (full_output, BassKernelResults)."""
    from concourse.bass_utils import run_bass_kernel_spmd
    nc = _get_nc()
    in_maps = _host_prep(inputs["x"], inputs["w_conv"], inputs["T"],
                         inputs["w_deconv"])
    res = run_bass_kernel_spmd(nc, in_maps, list(range(N_CORES)), trace=trace,
                               **(trace_kwargs or {}))
    x = np.asarray(inputs["x"], dtype=np.float32)
    full = np.empty((B, IN_FLT + OC, N, N), np.float32)
    full[:, :IN_FLT] = x
    for c in range(N_CORES):
        ya = res.results[c]["y"].astype(np.float32)   # [row, oc, v, j, b]
        # full[b, 128+oc, 8c+row, 4j+v] = ya[row, oc, v, j, b]
        yh = np.transpose(ya, (4, 1, 0, 3, 2)).reshape(B, OC, 8, N)
        full[:, IN_FLT:, 8 * c:8 * (c + 1), :] = yh
    return full, res


def kernel(**inputs) -> np.ndarray:
    out, _ = run(inputs, trace=False)
    return out


# revision 77
# speedup vs baseline: 1.0427x; 1.0065x over previous
"""Trainium2 Bass kernel for MinibatchDiscrimination2d.

Full computation:
  x (32,128,64,64) --conv s4--> x_r (32,3,16,16)
  M = x_r @ T  -> (32, 8192, 16)
  dist[b1,b2,d] = sum_f |M[b1,d,f]-M[b2,d,f]|
  out[b,d] = sum_b2 exp(-dist) - 1 -> (32,32,16,16)
  out_a = deconv s4 (32,32,64,64); return concat([x, out_a], ch)

Split host/device: the tiny front-end (strided conv -> x_r, 32x768 = 98KB,
~0.1% of total FLOPs) and the x passthrough run on the host; the device
kernel does everything that dominates: streaming the 100M-parameter T, the
M matmul, the B^2-coupled pairwise-L1/exp stage, and the deconv.

Sharding over 8 cores: split the t*t=256 output spatial positions of the
D_OUT axis into 8 row-bands (2 of 16 t-rows per core). Each core gets a
(768, 1024, 16) slice of T (fp8 e4m3, x128 scale) and computes M/dist/out
for its band for ALL 32 samples (no cross-core coupling anywhere), then
deconvs its band into 8 of the 64 output rows.

fp8: T scaled x128, x_r scaled x2 -> M psum = 256*M, descaled on the
PSUM->SBUF copy. The M matmul uses DoubleRow (2 contraction rows per
partition, 768 = 3 pairs of 128).

Engine layout per (dgroup g, pair-chunk pc):
  M_b   = x_r @ T_g (TensorE, fp8 DR)        -> PSUM -> Mb bf16 (ACT copy)
  D     = sgn^T @ M_b (TensorE, zero-padded to 128 contraction rows so the
          PE array shows full cell activity and the HAM clock gate stays
          at 2.4 GHz)                        -> PSUM (128 pairs, (s, f))
  dist  = reduce_|.|_f(D) (DVE, the pacing stage: PSUM f32 reads are
          capped at 1x = ~110 G elem/s, ~78us total)
  E     = exp(-dist) (ACT), acc_g = E^T @ inc (TensorE)
Deconv: acc (f32) x block-diagonal wd (f32) per (v, q), contiguous stores.

Per-core d index:  s = (r*16 + j)*32 + ch   (r in 0..1, j in 0..15, ch in 0..31)
dgroup g = s // 128; partition p = s % 128 = (rj%4)*32 + ch.
"""

import numpy as np
import ml_dtypes

N_CORES = 8
B, IN_FLT, N = 32, 128, 64
K = 4
T_SP = 16
OC = 32
F = 16
D_IN = 768
DSH = 1024                 # d per core
NG = DSH // 128            # 8 dgroups
KP = 3                     # contraction pairs (768 = 3 * 256) for DoubleRow
T_SCALE = 128.0
XR_SCALE = 2.0
M_DESCALE = 1.0 / (T_SCALE * XR_SCALE)

_CACHE = {}


def _build_nc():
    import concourse.bacc as bacc
    import concourse.mybir as mybir
    import concourse.tile as tile

    f32 = mybir.dt.float32
    bf16 = mybir.dt.bfloat16
    fp8 = mybir.dt.float8e4
    AFT = mybir.ActivationFunctionType
    ALU = mybir.AluOpType
    DR = mybir.MatmulPerfMode.DoubleRow

    nc = bacc.Bacc("TRN2", target_bir_lowering=False, debug=False,
                   num_devices=N_CORES)

    # host-packed inputs (see _host_prep for layouts)
    xrT = nc.dram_tensor("xrT", [128, 6 * B], fp8, kind="ExternalInput")
    tsh = nc.dram_tensor("tsh", [NG * KP * 128, 4096], fp8, kind="ExternalInput")
    # wd as 16 zero-padded (128,128) blocks [q, v]: block (q,v) is nonzero
    # only on partitions [32q, 32q+32), so the deconv can contract acc's
    # q-th partition block using a full-128-partition matmul.
    wd = nc.dram_tensor("wd", [128, 2048], f32, kind="ExternalInput")
    # sgn zero-padded to 128 contraction rows (see engine notes above)
    sgn = nc.dram_tensor("sgn", [128, 512], bf16, kind="ExternalInput")
    inc = nc.dram_tensor("inc", [128, 128], bf16, kind="ExternalInput")
    # y layout [row(4r+u), oc, v, j, b]: deconv PSUM->SBUF copies and the
    # final stores are fully contiguous (one DMA per row); host untangles.
    y = nc.dram_tensor("y", [8, OC, 4, T_SP, B], bf16, kind="ExternalOutput")

    with tile.TileContext(nc) as tc:
        with tc.tile_pool(name="const", bufs=1) as constp, \
             tc.tile_pool(name="Tp", bufs=24) as Tp, \
             tc.tile_pool(name="work", bufs=2) as wp, \
             tc.tile_pool(name="persist", bufs=1) as pp, \
             tc.tile_pool(name="psb", bufs=2, space="PSUM") as psb, \
             tc.tile_pool(name="ps_m", bufs=2, space="PSUM") as ps_m, \
             tc.tile_pool(name="ps_acc", bufs=1, space="PSUM") as ps_acc, \
             tc.tile_pool(name="ps_junk", bufs=1, space="PSUM") as ps_junk:

            sgn_sb = constp.tile([128, 512], bf16)
            nc.scalar.dma_start(sgn_sb[:], sgn[:])
            inc_sb = constp.tile([128, 128], bf16)
            nc.scalar.dma_start(inc_sb[:], inc[:])
            xrT_sb = constp.tile([128, 6 * B], fp8)
            nc.scalar.dma_start(xrT_sb[:], xrT[:])
            wd_sb = constp.tile([128, 2048], f32)
            nc.scalar.dma_start(wd_sb[:], wd[:])

            # junk-matmul helpers: full 128x128 array activity keeps the PE
            # HAM activity monitor above its warm threshold so the clock gate
            # stays at 2.4 GHz (it watches array cell activity, not busy%).
            # Outputs go to a dedicated PSUM bank that is never read.
            jpsum = ps_junk.tile([128, 512], f32, tag="junk")

            def _junk(n=1):
                for _ in range(n):
                    nc.tensor.matmul(jpsum[:], inc_sb[:],
                                     sgn_sb[:], start=True, stop=True)

            # Mb double-buffer: persistent 128-partition tiles whose rows
            # 32..127 stay zero forever (psD contraction zero-padding).
            Mbs = [pp.tile([128, 2048], bf16, tag=f"Mb{i}", name=f"Mb{i}")
                   for i in range(2)]
            for mb in Mbs:
                for q in range(1, 4):
                    nc.gpsimd.memset(mb[q * 32:(q + 1) * 32, :], 0.0)
            _junk(8)    # warm the PE before the pipeline fills (~3.4us is
                        # enough for the HAM window; more only delays g0)

            xrT_r = xrT_sb[:].rearrange("p (c two b) -> p c two b", c=KP, two=2)

            acc = pp.tile([128, NG * B], f32)        # col = g*32 + b
            wd_v = wd_sb[:].rearrange("p (q v m) -> p q v m", q=4, v=4)

            def _deconv_r(r):
                # deconv reads acc directly (f32 x f32 matmul, no staging):
                # out[(u,oc), (g,q,b)] = wd_blk(q,v)^T @ acc[:, (g,b)]
                # with j = g*4 + q; one matmul per (v, q).
                yst = wp.tile([128, 2048], bf16, tag="yst")  # cols (v, j, b)
                for v in range(4):
                    if r == 0:   # mid-loop: keep clear of the hot psD slots
                        psdt = ps_junk.tile([128, 512], f32, tag="junk",
                                            name=f"psd_{r}_{v}")
                    else:        # tail: psD pipeline is idle, use its slots
                        psdt_t = psb.tile([128, 1024], f32, tag="big",
                                          name=f"psd_{r}_{v}")
                        psdt = psdt_t[:, :512]
                    psd_q = psdt[:].rearrange("p (g q b) -> p g q b", g=4, q=4)
                    for q in range(4):
                        nc.tensor.matmul(
                            psd_q[:, :, q, :],
                            wd_v[:, q, v],
                            acc[:, 4 * r * B:(4 * r + 4) * B],
                            start=True, stop=True)
                    if r == 1 and v % 2 == 0:
                        nc.vector.tensor_copy(yst[:, v * 512:(v + 1) * 512],
                                              psdt[:])
                    else:
                        nc.scalar.copy(yst[:, v * 512:(v + 1) * 512], psdt[:])
                for u in range(4):
                    nc.sync.dma_start(
                        y[4 * r + u],
                        yst[u * 32:(u + 1) * 32, :]
                        .rearrange("o (v j b) -> o v j b", v=4, j=T_SP))

            # E-accumulation is emitted one pc-slot late so the acc matmul
            # (which depends on its own pc's reduce->exp chain) never sits in
            # the in-order tensor queue ahead of the next psD group.
            pend = []
            accg_box = [None]

            def _flush_acc():
                if not pend:
                    return
                fg, fpc, fE = pend.pop(0)
                if fpc == 0:
                    accg_box[0] = ps_acc.tile([128, B], f32, tag="accg",
                                              name=f"accg{fg}")
                nc.tensor.matmul(
                    accg_box[0][:], fE[:], inc_sb[:, fpc * B:(fpc + 1) * B],
                    start=(fpc == 0), stop=(fpc == 3))
                if fpc == 3:
                    nc.scalar.copy(acc[:, fg * B:(fg + 1) * B], accg_box[0][:])
                    if fg in (NG // 2 - 1, NG - 1):
                        _deconv_r(fg // (NG // 2))

            # ---- main loop: stages fused per dgroup g
            for g in range(NG):
                Ts = []
                for kp in range(KP):
                    Tt = Tp.tile([128, 4096], fp8, tag="T")
                    row = (g * KP + kp) * 128
                    nc.sync.dma_start(Tt[:], tsh[row:row + 128, :])
                    Ts.append(Tt)
                Mb = Mbs[g % 2]                      # (128, (s, f)); rows 32+ zero
                for ncn in range(4):
                    psm = ps_m.tile([B, 512], f32, tag="mm")
                    for kp in range(KP):
                        nc.tensor.matmul(
                            psm[:], xrT_r[:, kp],
                            Ts[kp][:].rearrange("p (two n) -> p two n", two=2)
                            [:, :, ncn * 512:(ncn + 1) * 512],
                            start=(kp == 0), stop=(kp == KP - 1), perf_mode=DR)
                    nc.scalar.mul(Mb[:B, ncn * 512:(ncn + 1) * 512], psm[:], M_DESCALE)
                for pc in range(4):
                    dist = wp.tile([128, 128], f32, tag="dist")
                    for nh in range(2):
                        psD_t = psb.tile([128, 1024], f32, tag="big")
                        for nq in range(2):
                            ncn = nh * 2 + nq
                            nc.tensor.matmul(
                                psD_t[:, nq * 512:(nq + 1) * 512],
                                sgn_sb[:, pc * 128:(pc + 1) * 128],
                                Mb[:, ncn * 512:(ncn + 1) * 512],
                                start=True, stop=True)
                        if nh == 0:
                            _flush_acc()   # previous pc's E-accumulation
                        nc.vector.tensor_reduce(
                            dist[:, nh * 64:(nh + 1) * 64],
                            psD_t[:].rearrange("p (s f) -> p s f", f=F),
                            axis=mybir.AxisListType.X, op=ALU.add,
                            apply_absolute_value=True)
                    Egp = wp.tile([128, 128], bf16, tag="E")
                    nc.scalar.activation(Egp[:], dist[:], AFT.Exp, scale=-1.0)
                    pend.append((g, pc, Egp))
                    _junk(2)
            _flush_acc()   # final (g=7, pc=3) accumulation + copy + deconv

    nc.finalize()
    return nc


def _host_prep(x, w_conv, T, w_deconv):
    """Host front-end (conv -> x_r, fp8 packing) + the 8 per-core inputs."""
    bf = ml_dtypes.bfloat16
    f8 = ml_dtypes.float8_e4m3

    def e4(v):
        return np.clip(v, -240.0, 240.0).astype(f8)

    # strided conv on host: x_r[b, o, i, j] (tiny: 32x3x16x16)
    xp = np.asarray(x, np.float32).reshape(B, IN_FLT, T_SP, K, T_SP, K)
    xr = np.einsum('bcirjs,ocrs->boij', xp, np.asarray(w_conv, np.float32),
                   optimize=True)
    # xrT[p, k*32 + b] = 2 * x_r[b, k*128 + p]  (fp8, matches T's x128 scale)
    xrf = xr.reshape(B, D_IN) * XR_SCALE
    xrT_host = e4(np.ascontiguousarray(xrf.T.reshape(6, 128, B)
                                       .transpose(1, 0, 2).reshape(128, 6 * B)))

    # deconv weights: block (q, v) nonzero only on partitions [32q, 32q+32),
    # wd_host[32q+ic, q, v, u*32+oc] = w_deconv[oc, ic, u, v]
    wd0 = np.transpose(w_deconv, (1, 3, 2, 0)).reshape(OC, 4, 128)  # [c, v, m]
    wdq = np.zeros((128, 4, 4, 128), np.float32)
    for q in range(4):
        wdq[32 * q:32 * (q + 1), q] = wd0
    wd_host = wdq.reshape(128, 2048)

    # pairwise sign matrix (b1 < b2, 496 pairs padded to 512) and incidence
    # (zero-padded to 128 contraction rows for full PE activity)
    pairs = [(a, b) for a in range(B) for b in range(a + 1, B)]
    sgn_host = np.zeros((128, 512), np.float32)
    inc_host = np.zeros((128, 128), np.float32)
    for p, (a, b) in enumerate(pairs):
        sgn_host[a, p] = 1.0
        sgn_host[b, p] = -1.0
        inc_host[p % 128, (p // 128) * B + a] = 1.0
        inc_host[p % 128, (p // 128) * B + b] = 1.0
    sgn_host = sgn_host.astype(bf)
    inc_host = inc_host.astype(bf)

    Tq = e4(np.asarray(T, np.float32) * T_SCALE).reshape(
        D_IN, OC, T_SP, T_SP, F)

    in_maps = []
    for c in range(N_CORES):
        # T shard: i rows 2c, 2c+1; column order s=(r*16+j)*32+ch, then f
        tslice = Tq[:, :, 2 * c:2 * c + 2, :, :]            # (768, ch, r, j, f)
        tshard = np.ascontiguousarray(
            np.transpose(tslice, (0, 2, 3, 1, 4)).reshape(D_IN, DSH * F))
        # DoubleRow pack: rows (g, kp, p), cols (two, n)
        t3 = tshard.reshape(KP, 2, 128, NG, 2048)
        tpk = np.ascontiguousarray(t3.transpose(3, 0, 2, 1, 4)).reshape(
            NG * KP * 128, 4096)
        in_maps.append({
            "xrT": xrT_host,
            "tsh": tpk,
            "wd": wd_host,
            "sgn": sgn_host,
            "inc": inc_host,
        })
    return in_maps


def _get_nc():
    if "nc" not in _CACHE:
        _CACHE["nc"] = _build_nc()
    return _CACHE["nc"]


def run(inputs, trace=False, trace_kwargs=None):
    """Run on hardware; returns (full_output, BassKernelResults)."""
    from concourse.bass_utils import run_bass_kernel_spmd
    nc = _get_nc()
    in_maps = _host_prep(inputs["x"], inputs["w_conv"], inputs["T"],
                         inputs["w_deconv"])
    res = run_bass_kernel_spmd(nc, in_maps, list(range(N_CORES)), trace=trace,
                               **(trace_kwargs or {}))
    x = np.asarray(inputs["x"], dtype=np.float32)
    full = np.empty((B, IN_FLT + OC, N, N), np.float32)
    full[:, :IN_FLT] = x
    for c in range(N_CORES):
        ya = res.results[c]["y"].astype(np.float32)   # [row, oc, v, j, b]
        # full[b, 128+oc, 8c+row, 4j+v] = ya[row, oc, v, j, b]
        yh = np.transpose(ya, (4, 1, 0, 3, 2)).reshape(B, OC, 8, N)
        full[:, IN_FLT:, 8 * c:8 * (c + 1), :] = yh
    return full, res


def kernel(**inputs) -> np.ndarray:
    out, _ = run(inputs, trace=False)
    return out
